# revision 2
# baseline (speedup 1.0000x reference)
"""Trainium2 Bass kernel v2 for nn_EnhancementGenerator.

Same math reformulation as the baseline (non-recurrent bidirectional 2-layer
GRU + KANLinear + 1.2*sigmoid), restructured around fp8-e4m3 DoubleRow
matmuls (one instruction contracts 2x128 K-subtiles; measured 213ns at N=500,
the same as one fp16 K<=128 matmul, i.e. 2x throughput):

* r/z gate input projections run as fp8 DR (K=256 in 1 instr). The n-gates
  stay fp16: host simulation puts the hybrid at 8.9e-3 rel err vs 2.0e-2 for
  all-fp8 (tanh path dominates the error budget).
* KAN: exact 7-block truncated-power basis, but the outer (+-0.6) kink
  blocks -- whose contribution std is ~0.12 vs 0.65 total, so fp8 noise on
  them is negligible -- are packed as one fp8 DR pair per M-chunk. The
  out-256 row's f^4/f^5 deg-5 blocks pair the same way.
* silu base branch folded into the full 8-fn spline basis (error ~1e-4)
  instead of a cubic fit.
* Kink relus computed as min/max tensor_scalar ops on DVE/Pool, freeing
  Scalar; all output DMAs issue from the Sync queue.

Matmul instructions per half-sample stream: 31 vs baseline 38.
"""
import os
import sys

for _p in (
    "/root/.axon_site",
    "/root/.axon_site/_ro/trn_rl_repo",
    "/root/.axon_site/_ro/pypackages",
    "/opt/trn_rl_repo",
    "/opt/pypackages",
):
    if os.path.isdir(_p) and _p not in sys.path:
        sys.path.append(_p)

import numpy as np
import ml_dtypes

import concourse.bass as bass
import concourse.tile as tile
from concourse import bacc, mybir
from concourse.bass_utils import run_bass_kernel_spmd

F32 = mybir.dt.float32
FP16 = mybir.dt.float16
FP8 = mybir.dt.float8e4
AF = mybir.ActivationFunctionType
ALU = mybir.AluOpType
DR = mybir.MatmulPerfMode.DoubleRow

N_CORES = 8
B, T, IN_SIZE, HID, OUT_SIZE = 64, 1000, 257, 40, 257
NT = 500
SPB = B // N_CORES
ROWS = SPB * T
OPAD = 264
PG = 104            # f at 0:40, b at 64:104
BO = 64
KCH = [(0, 128), (128, 128), (256, 8)]


# --------------------------------------------------------------------------
# host-side weight folding
# --------------------------------------------------------------------------
def fold_weights(inp):
    from math import comb
    W = {k: np.asarray(v, dtype=np.float64) for k, v in inp.items()}
    out = {}

    # ---- fp8 DR weights, r/z gates: [128, 2, 4*128] (blocks l0r,l0z,l1r,l1z)
    wdr = np.zeros((128, 2, 4 * 128))
    for l in range(2):
        for g in range(2):
            blk = l * 2 + g
            for d, Wih in ((0, W["Wih_f"]), (BO, W["Wih_b"])):
                wm = Wih[l][g * 40:(g + 1) * 40]            # (40, 257)
                wdr[:, 0, blk * 128 + d:blk * 128 + d + 40] = wm[:, 0:128].T
                wdr[:, 1, blk * 128 + d:blk * 128 + d + 40] = wm[:, 128:256].T
    out["wdr"] = wdr.reshape(128, 2 * 4 * 128)

    # ---- K=8 fp16 chunk (x rows 256:264): x256 for l0r, l0z
    wc2 = np.zeros((8, 2 * 128))
    for g in range(2):
        wc2[0, g * 128 + 0:g * 128 + 40] = W["Wih_f"][0][g * 40:(g + 1) * 40, 256]
        wc2[0, g * 128 + BO:g * 128 + BO + 40] = \
            W["Wih_b"][0][g * 40:(g + 1) * 40, 256]
    out["wc2"] = wc2

    # ---- fp16 n-gate weights per K chunk: [ksz, 2*128] (l0n, l1n)
    for ci, (k0, ksz) in enumerate(KCH):
        wn = np.zeros((ksz, 2 * 128))
        for l in range(2):
            for d, Wih in ((0, W["Wih_f"]), (BO, W["Wih_b"])):
                wpad = np.zeros((40, OPAD))
                wpad[:, :IN_SIZE] = Wih[l][80:120]
                wn[:, l * 128 + d:l * 128 + d + 40] = wpad[:, k0:k0 + ksz].T
        out[f"wgn{ci}"] = wn

    # ---- gh (negated blockdiag) [106, 3*128]; row 104 = x256 for r2/z2
    wgh = np.zeros((106, 3 * 128))
    for g in range(3):
        wgh[0:40, g * 128:g * 128 + 40] = -W["Whh_f"][1][g * 40:(g + 1) * 40].T
        wgh[BO:BO + 40, g * 128 + BO:g * 128 + BO + 40] = \
            -W["Whh_b"][1][g * 40:(g + 1) * 40].T
    for g in range(2):
        wgh[104, g * 128 + 0:g * 128 + 40] = W["Wih_f"][1][g * 40:(g + 1) * 40, 256]
        wgh[104, g * 128 + BO:g * 128 + BO + 40] = \
            W["Wih_b"][1][g * 40:(g + 1) * 40, 256]
    out["wgh"] = wgh

    # ---- gru biases [104, 8]
    bg = np.zeros((PG, 8))
    for l in range(2):
        for gi_ in range(2):
            bg[0:40, l * 4 + gi_] = (W["bih_f"][l][gi_ * 40:(gi_ + 1) * 40]
                                     + W["bhh_f"][l][gi_ * 40:(gi_ + 1) * 40])
            bg[BO:BO + 40, l * 4 + gi_] = (W["bih_b"][l][gi_ * 40:(gi_ + 1) * 40]
                                           + W["bhh_b"][l][gi_ * 40:(gi_ + 1) * 40])
        bg[0:40, l * 4 + 2] = W["bhh_f"][l][80:120]
        bg[BO:BO + 40, l * 4 + 2] = W["bhh_b"][l][80:120]
        bg[0:40, l * 4 + 3] = W["bih_f"][l][80:120]
        bg[BO:BO + 40, l * 4 + 3] = W["bih_b"][l][80:120]
    out["bgru"] = bg

    # ---- KAN: truncated-power basis with silu folded in
    h = 0.4
    t = -2.2 + h * np.arange(12)
    w = W["spline_weight"] * W["spline_scaler"][..., None]
    s = np.zeros((8, 12))
    for m in range(8):
        for k in range(5):
            s[m, m + k] = ((-1) ** k) * comb(4, k) / (6 * h ** 3)
    V = np.einsum("oim,mj->oij", w, s)                       # (257, 80, 12)
    A = np.zeros((4, 257, 80))
    for j in range(6):
        for d in range(4):
            A[d] += V[:, :, j] * comb(3, d) * ((-t[j]) ** (3 - d))
    # device block convention: pL6 = min(f+.6,0)^3, pL2 = min(f+.2,0)^3,
    # pR2 = relu(f-.2)^3, pR6 = relu(f-.6)^3; weights:
    Vk = [-V[:, :, 4], -V[:, :, 5], V[:, :, 6], V[:, :, 7]]

    # silu fit in the full 8-fn basis
    fgrid = np.linspace(-1.0, 1.0, 4001)
    bas = [np.ones_like(fgrid), fgrid, fgrid ** 2, fgrid ** 3,
           np.minimum(fgrid + 0.6, 0.0) ** 3,
           np.minimum(fgrid + 0.2, 0.0) ** 3,
           np.maximum(fgrid - 0.2, 0.0) ** 3,
           np.maximum(fgrid - 0.6, 0.0) ** 3]
    M8 = np.stack(bas, 1)
    csil, *_ = np.linalg.lstsq(M8, fgrid / (1 + np.exp(-fgrid)), rcond=None)
    bw = W["base_weight"]
    for d in range(4):
        A[d] = A[d] + csil[d] * bw
    for j in range(4):
        Vk[j] = Vk[j] + csil[4 + j] * bw

    slope = W["slope"]

    # fp16 blocks (f, s2, s3, pL2, pR2) for M chunks 0:128, 128:256
    wkan = np.zeros((PG, 5 * 256))
    for idx, m in enumerate([A[1].T, A[2].T, A[3].T, Vk[1].T, Vk[2].T]):
        ms = m * slope[None, :]                              # (80, 257)
        wkan[0:40, idx * 256:idx * 256 + 256] = ms[0:40, 0:256]
        wkan[BO:BO + 40, idx * 256:idx * 256 + 256] = ms[40:80, 0:256]
    out["wkan"] = wkan

    # fp8 DR pair (pL6, pR6) for M chunks 0:128, 128:256: [104, 2, 256]
    wk67 = np.zeros((PG, 2, 256))
    for sub, m in enumerate([Vk[0].T, Vk[3].T]):
        ms = m * slope[None, :]
        wk67[0:40, sub, :] = ms[0:40, 0:256]
        wk67[BO:BO + 40, sub, :] = ms[40:80, 0:256]
    out["wk67"] = wk67.reshape(PG, 2 * 256)

    # out-256 row: same 7-block basis, M=64-padded (DR needs M in {64,128}):
    # fp16 blocks (f, s2, s3, pL2, pR2) at [104, 5*64] (col 0 real)
    wkm2 = np.zeros((PG, 5 * 64))
    for idx, m in enumerate([A[1], A[2], A[3], Vk[1], Vk[2]]):
        ms = m[256] * slope[256]                             # (80,)
        wkm2[0:40, idx * 64] = ms[0:40]
        wkm2[BO:BO + 40, idx * 64] = ms[40:80]
    out["wkm2"] = wkm2
    # fp8 DR pair (pL6, pR6) row-256: [104, 2, 64]
    wk45 = np.zeros((PG, 2, 64))
    for sub, m in enumerate([Vk[0], Vk[3]]):
        ms = m[256] * slope[256]
        wk45[0:40, sub, 0] = ms[0:40]
        wk45[BO:BO + 40, sub, 0] = ms[40:80]
    out["wk45"] = wk45.reshape(PG, 2 * 64)

    bk = np.zeros((128, 3))
    a0 = A[0].sum(axis=1) * slope
    bk[0:128, 0] = a0[0:128]
    bk[0:128, 1] = a0[128:256]
    bk[0, 2] = a0[256]
    out["bkan"] = bk
    return {k: np.ascontiguousarray(v, dtype=np.float32) for k, v in out.items()}


# --------------------------------------------------------------------------
# device kernel
# --------------------------------------------------------------------------
def build_nc(n_samples=SPB):
    rows = n_samples * T
    NT2 = 2 * NT
    nc = bacc.Bacc("TRN2", target_bir_lowering=False, debug=False)
    mm = nc.tensor.matmul

    def rr2(ap):
        return ap.rearrange("p (a b) -> p a b", a=2)

    xdr_d = nc.dram_tensor("xdr", [128, n_samples * 2000], FP8, kind="ExternalInput")
    xt_d = nc.dram_tensor("xt", [OPAD, rows], FP16, kind="ExternalInput")
    wdr_d = nc.dram_tensor("wdr", [128, 2 * 4 * 128], FP8, kind="ExternalInput")
    wc2_d = nc.dram_tensor("wc2", [8, 2 * 128], FP16, kind="ExternalInput")
    wgn_d = [nc.dram_tensor(f"wgn{ci}", [ksz, 2 * 128], FP16, kind="ExternalInput")
             for ci, (k0, ksz) in enumerate(KCH)]
    wgh_d = nc.dram_tensor("wgh", [106, 3 * 128], FP16, kind="ExternalInput")
    wkan_d = nc.dram_tensor("wkan", [PG, 5 * 256], FP16, kind="ExternalInput")
    wk67_d = nc.dram_tensor("wk67", [PG, 2 * 256], FP8, kind="ExternalInput")
    wkm2_d = nc.dram_tensor("wkm2", [PG, 5 * 64], FP16, kind="ExternalInput")
    wk45_d = nc.dram_tensor("wk45", [PG, 2 * 64], FP8, kind="ExternalInput")
    bgru_d = nc.dram_tensor("bgru", [PG, 8], F32, kind="ExternalInput")
    bkan_d = nc.dram_tensor("bkan", [128, 3], F32, kind="ExternalInput")
    yt_d = nc.dram_tensor("yt", [OPAD, rows], FP16, kind="ExternalOutput")

    with tile.TileContext(nc) as tc:
        with (
            tc.tile_pool(name="wts", bufs=1) as wp,
            tc.tile_pool(name="xin", bufs=3) as xp,
            tc.tile_pool(name="work", bufs=1) as kp,
            tc.tile_pool(name="outp", bufs=2) as op_,
            tc.tile_pool(name="psg", bufs=1, space="PSUM") as psg,
            tc.tile_pool(name="psk", bufs=1, space="PSUM") as psk,
        ):
            # ---- resident weights
            wdr_sb = wp.tile([128, 2, 4 * 128], FP8, tag="wdr")
            nc.sync.dma_start(wdr_sb[:], rr2(wdr_d[:]))
            wc2_sb = wp.tile([8, 2 * 128], FP16, tag="wc2")
            nc.sync.dma_start(wc2_sb[:], wc2_d[:])
            wgn_sb = []
            for ci, (k0, ksz) in enumerate(KCH):
                wt = wp.tile([ksz, 2 * 128], FP16, tag=f"wgn{ci}")
                nc.sync.dma_start(wt[:], wgn_d[ci][:])
                wgn_sb.append(wt)
            wgh_sb = wp.tile([106, 3 * 128], FP16, tag="wgh")
            nc.sync.dma_start(wgh_sb[:], wgh_d[:])
            wkan_sb = wp.tile([PG, 5 * 256], FP16, tag="wkan")
            nc.sync.dma_start(wkan_sb[:], wkan_d[:])
            wk67_sb = wp.tile([PG, 2, 256], FP8, tag="wk67")
            nc.sync.dma_start(wk67_sb[:], rr2(wk67_d[:]))
            wkm2_sb = wp.tile([PG, 5 * 64], FP16, tag="wkm2")
            nc.sync.dma_start(wkm2_sb[:], wkm2_d[:])
            wk45_sb = wp.tile([PG, 2, 64], FP8, tag="wk45")
            nc.sync.dma_start(wk45_sb[:], rr2(wk45_d[:]))
            bg = wp.tile([PG, 8], F32, tag="bgru")
            nc.sync.dma_start(bg[:], bgru_d[:])
            bk = wp.tile([128, 3], F32, tag="bkan")
            nc.sync.dma_start(bk[:], bkan_d[:])

            # PE warm-up (p-state ramp) against resident fp8 weights
            wmp = psg.tile([128, NT], F32, tag="psA", name="warm")
            for _ in range(14):
                mm(wmp[:], wdr_sb[:, :, 0:128], wdr_sb[:, :, 0:NT],
                   start=True, stop=True, perf_mode=DR)

            S = [dict() for _ in range(n_samples)]

            def load_x(smp):
                st = S[smp]
                s0 = smp * T
                xdr = xp.tile([128, 2, T], FP8, tag="xdr", name="xdr")
                nc.sync.dma_start(
                    xdr[:], rr2(xdr_d[:, smp * 2000:(smp + 1) * 2000]))
                st["xdr"] = xdr
                st["xs"] = []
                for ci, (k0, ksz) in enumerate(KCH):
                    xtile = xp.tile([ksz, NT2], FP16, tag=f"x{ci}", name=f"x{ci}")
                    nc.sync.dma_start(xtile[:], xt_d[k0:k0 + ksz, s0:s0 + NT2])
                    st["xs"].append(xtile)

            def emit_l0(smp):
                st = S[smp]
                xdr, xs = st["xdr"], st["xs"]
                rt = kp.tile([PG, NT2], FP16, tag="rt", bufs=2)
                zt = kp.tile([PG, NT2], FP16, tag="zt", bufs=2)
                ut = kp.tile([PG, NT2], FP16, tag="ut", bufs=2)
                H = [slice(0, NT), slice(NT, NT2)]
                for g, dst in ((0, rt), (1, zt)):
                    ps = [psg.tile([128, NT], F32, tag="psA", name=f"l0g{g}h0"),
                          psg.tile([128, NT], F32, tag="psB", name=f"l0g{g}h1")]
                    c0 = g * 128
                    for h in range(2):
                        mm(ps[h][:], wdr_sb[:, :, c0:c0 + 128], xdr[:, :, H[h]],
                           start=True, stop=False, perf_mode=DR)
                    for h in range(2):
                        mm(ps[h][:], wc2_sb[:, c0:c0 + 128], xs[2][:, H[h]],
                           start=False, stop=True)
                    for h in range(2):
                        nc.scalar.activation(dst[:, H[h]], ps[h][0:PG, :],
                                             AF.Sigmoid, bias=bg[:, g:g + 1])
                    yield
                psn = [psg.tile([128, NT], F32, tag="psA", name="l0nh0"),
                       psg.tile([128, NT], F32, tag="psB", name="l0nh1")]
                for ci in range(3):
                    for h in range(2):
                        mm(psn[h][:], wgn_sb[ci][:, 0:128], xs[ci][:, H[h]],
                           start=(ci == 0), stop=(ci == 2))
                for h in range(2):
                    nc.vector.scalar_tensor_tensor(
                        ut[:, H[h]], rt[:, H[h]], bg[:, 2:3], psn[h][0:PG, :],
                        op0=ALU.mult, op1=ALU.add)
                yield
                n1 = kp.tile([PG, NT2], FP16, tag="n1", bufs=2)
                nc.scalar.activation(n1[:], ut[:], AF.Tanh, bias=bg[:, 3:4])
                h1n = kp.tile([106, NT2], FP16, tag="h1n", bufs=2)
                nc.vector.scalar_tensor_tensor(
                    h1n[0:PG, :], zt[:], 1.0, n1[:], op0=ALU.subtract, op1=ALU.mult)
                s0 = smp * T
                nc.sync.dma_start(h1n[PG:PG + 2, :], xt_d[256:258, s0:s0 + NT2])
                st["h1n"] = h1n

            def emit_l1(smp):
                st = S[smp]
                xdr, xs, h1n = st["xdr"], st["xs"], st["h1n"]
                r2t = kp.tile([PG, NT2], FP16, tag="r2t")
                z2t = kp.tile([PG, NT2], FP16, tag="z2t")
                t2t = kp.tile([PG, NT2], FP16, tag="t2t")
                vt = kp.tile([PG, NT2], FP16, tag="vt")
                H = [slice(0, NT), slice(NT, NT2)]
                for g, dst in ((0, r2t), (1, z2t)):
                    ps = [psg.tile([128, NT], F32, tag="psE", name=f"l1g{g}h0"),
                          psg.tile([128, NT], F32, tag="psF", name=f"l1g{g}h1")]
                    c0 = (2 + g) * 128
                    for h in range(2):
                        mm(ps[h][:], wdr_sb[:, :, c0:c0 + 128], xdr[:, :, H[h]],
                           start=True, stop=False, perf_mode=DR)
                    for h in range(2):
                        mm(ps[h][:], wgh_sb[:, g * 128:g * 128 + 128], h1n[:, H[h]],
                           start=False, stop=True)
                    for h in range(2):
                        nc.scalar.activation(dst[:, H[h]], ps[h][0:PG, :],
                                             AF.Sigmoid, bias=bg[:, 4 + g:5 + g])
                    yield
                psn = [psg.tile([128, NT], F32, tag="psE", name="l1nh0"),
                       psg.tile([128, NT], F32, tag="psF", name="l1nh1")]
                for ci in range(3):
                    for h in range(2):
                        mm(psn[h][:], wgn_sb[ci][:, 128:256], xs[ci][:, H[h]],
                           start=(ci == 0), stop=(ci == 2))
                yield
                psp = [psg.tile([128, NT], F32, tag="psC", name="l1ph0"),
                       psg.tile([128, NT], F32, tag="psG", name="l1ph1")]
                for h in range(2):
                    mm(psp[h][:], wgh_sb[:, 256:384], h1n[:, H[h]],
                       start=True, stop=True)
                for h in range(2):
                    nc.vector.scalar_tensor_tensor(
                        t2t[:, H[h]], psp[h][0:PG, :], bg[:, 6:7], r2t[:, H[h]],
                        op0=ALU.add, op1=ALU.mult)
                    nc.vector.tensor_add(vt[:, H[h]], t2t[:, H[h]], psn[h][0:PG, :])
                yield
                n2 = kp.tile([PG, NT2], FP16, tag="n2")
                nc.scalar.activation(n2[:], vt[:], AF.Tanh, bias=bg[:, 7:8])
                dd = kp.tile([PG, NT2], FP16, tag="dd")
                nc.vector.tensor_add(dd[:], h1n[0:PG, :], n2[:])
                qq = kp.tile([PG, NT2], FP16, tag="qq")
                nc.vector.tensor_mul(qq[:], z2t[:], dd[:])
                feat = kp.tile([PG, NT2], FP16, tag="feat", bufs=2)
                nc.vector.tensor_sub(feat[0:64, :], n2[0:64, :], qq[0:64, :])
                nc.vector.tensor_sub(feat[BO:BO + 40, :], n2[BO:BO + 40, ::-1],
                                     qq[BO:BO + 40, ::-1])
                # s2 on Scalar (Square), s3 on DVE
                s2 = kp.tile([PG, NT2], FP16, tag="s2", bufs=2)
                nc.scalar.activation(s2[:], feat[:], AF.Square)
                s3 = kp.tile([PG, NT2], FP16, tag="s3", bufs=2)
                nc.vector.tensor_mul(s3[:], s2[:], feat[:])
                yield
                # NOTE: no gpsimd elementwise -- its q7 software ops saturate
                # SBUF and slow PE matmuls ~1.7x while running (measured).
                mL = kp.tile([PG, NT2], FP16, tag="mL")
                nc.vector.tensor_scalar(mL[:], feat[:], 0.2, 0.0, ALU.add, ALU.min)
                qL = kp.tile([PG, NT2], FP16, tag="qL")
                nc.vector.tensor_mul(qL[:], mL[:], mL[:])
                pL = kp.tile([PG, NT2], FP16, tag="pL", bufs=2)
                nc.vector.tensor_mul(pL[:], qL[:], mL[:])
                rR = kp.tile([PG, NT2], FP16, tag="rR")
                nc.vector.tensor_scalar(rR[:], feat[:], -0.2, 0.0, ALU.add, ALU.max)
                qR = kp.tile([PG, NT2], FP16, tag="qR")
                nc.vector.tensor_mul(qR[:], rR[:], rR[:])
                pR = kp.tile([PG, NT2], FP16, tag="pR", bufs=2)
                nc.vector.tensor_mul(pR[:], qR[:], rR[:])
                yield
                # outer kinks -> fp8 DR pair tile [104, 2, NT2]
                pk67 = kp.tile([PG, 2, NT2], FP8, tag="pk67", bufs=2)
                mL6 = kp.tile([PG, NT2], FP16, tag="mL6")
                nc.vector.tensor_scalar(mL6[:], feat[:], 0.6, 0.0, ALU.add, ALU.min)
                qL6 = kp.tile([PG, NT2], FP16, tag="qL6")
                nc.scalar.activation(qL6[:], mL6[:], AF.Square)
                nc.vector.tensor_mul(pk67[:, 0, :], qL6[:], mL6[:])
                rR6 = kp.tile([PG, NT2], FP16, tag="rR6")
                nc.vector.tensor_scalar(rR6[:], feat[:], -0.6, 0.0, ALU.add, ALU.max)
                qR6 = kp.tile([PG, NT2], FP16, tag="qR6")
                nc.scalar.activation(qR6[:], rR6[:], AF.Square)
                nc.vector.tensor_mul(pk67[:, 1, :], qR6[:], rR6[:])
                st["rhs"] = [feat, s2, s3, pL, pR]
                st["pk67"] = pk67
                yield

            def emit_kan(smp):
                st = S[smp]
                s0 = smp * T
                rhs_list = st["rhs"]
                pk67 = st["pk67"]
                H = [slice(0, NT), slice(NT, NT2)]
                for mc in range(2):
                    m0 = mc * 128
                    ot = op_.tile([128, NT2], FP16, tag=f"ot{mc}", name=f"ot{mc}")
                    po = [psk.tile([128, NT], F32, tag="kan", name=f"po{mc}h0",
                                   bufs=2),
                          psk.tile([128, NT], F32, tag="kan", name=f"po{mc}h1",
                                   bufs=2)]
                    for pi, r in enumerate(rhs_list):
                        for h in range(2):
                            mm(po[h][:], wkan_sb[:, pi * 256 + m0:pi * 256 + m0 + 128],
                               r[:, H[h]], start=(pi == 0), stop=False)
                        if pi == 2:
                            yield
                    for h in range(2):
                        mm(po[h][:], wk67_sb[:, :, m0:m0 + 128], pk67[:, :, H[h]],
                           start=False, stop=True, perf_mode=DR)
                    for h in range(2):
                        nc.scalar.activation(ot[:, H[h]], po[h][:], AF.Sigmoid,
                                             bias=bk[:, mc:mc + 1])
                    nc.sync.dma_start(yt_d[m0:m0 + 128, s0:s0 + NT2], ot[:])
                    yield
                # out row 256: same 7-block basis at M=64
                ot2 = op_.tile([8, NT2], FP16, tag="ot2", name="ot2")
                po = [psk.tile([128, NT], F32, tag="kan", name="po2h0", bufs=2),
                      psk.tile([128, NT], F32, tag="kan", name="po2h1", bufs=2)]
                for pi, r in enumerate(rhs_list):
                    for h in range(2):
                        mm(po[h][0:64, :], wkm2_sb[:, pi * 64:(pi + 1) * 64],
                           r[:, H[h]], start=(pi == 0), stop=False)
                    if pi == 2:
                        yield
                for h in range(2):
                    mm(po[h][0:64, :], wk45_sb[:], pk67[:, :, H[h]],
                       start=False, stop=True, perf_mode=DR)
                for h in range(2):
                    nc.scalar.activation(ot2[:, H[h]], po[h][0:8, :], AF.Sigmoid,
                                         bias=bk[0:8, 2:3])
                nc.sync.dma_start(yt_d[256:264, s0:s0 + NT2], ot2[:])
                yield

            load_x(0)
            if n_samples > 1:
                load_x(1)
            for k in range(n_samples + 2):
                if k + 2 < n_samples:
                    load_x(k + 2)
                gens = []
                if k < n_samples:
                    gens.append(emit_l0(k))
                if 0 <= k - 2 < n_samples:
                    gens.append(emit_kan(k - 2))
                if 0 <= k - 1 < n_samples:
                    gens.append(emit_l1(k - 1))
                # round-robin the streams so the PE queue always holds
                # independent matmuls while psums/rhs of one stream drain
                while gens:
                    done = []
                    for g in gens:
                        try:
                            next(g)
                        except StopIteration:
                            done.append(g)
                    for g in done:
                        gens.remove(g)
                if 0 <= k - 2 < n_samples:
                    S[k - 2].clear()
    nc.compile()
    return nc


# --------------------------------------------------------------------------
# host entry point
# --------------------------------------------------------------------------
_NC_CACHE = {}


def _get_nc(n_samples=SPB):
    if n_samples not in _NC_CACHE:
        _NC_CACHE[n_samples] = build_nc(n_samples)
    return _NC_CACHE[n_samples]


def make_in_maps(inputs, n_samples=SPB, n_cores=N_CORES):
    x = np.asarray(inputs["x"], dtype=np.float32)
    Wf = fold_weights(inputs)
    for k in ("wc2", "wgn0", "wgn1", "wgn2", "wgh", "wkan", "wkm2"):
        Wf[k] = np.ascontiguousarray(Wf[k].astype(np.float16))
    for k in ("wdr", "wk67", "wk45"):
        Wf[k] = np.ascontiguousarray(Wf[k].astype(ml_dtypes.float8_e4m3fn))
    in_maps = []
    for c in range(n_cores):
        xc = x[c * n_samples:(c + 1) * n_samples].reshape(n_samples * T, IN_SIZE)
        xt = np.zeros((OPAD, n_samples * T), dtype=np.float16)
        xt[:IN_SIZE] = xc.T.astype(np.float16)
        xcf8 = xc.T.astype(ml_dtypes.float8_e4m3fn)
        xdr = np.zeros((128, n_samples * 2000), dtype=ml_dtypes.float8_e4m3fn)
        for smp in range(n_samples):
            xdr[:, smp * 2000:smp * 2000 + 1000] = xcf8[0:128, smp * T:(smp + 1) * T]
            xdr[:, smp * 2000 + 1000:(smp + 1) * 2000] = \
                xcf8[128:256, smp * T:(smp + 1) * T]
        in_maps.append({"xt": np.ascontiguousarray(xt),
                        "xdr": np.ascontiguousarray(xdr), **Wf})
    return in_maps


def kernel(**inputs):
    x = np.asarray(inputs["x"], dtype=np.float32)
    assert x.shape == (B, T, IN_SIZE), x.shape
    nc = _get_nc(SPB)
    in_maps = make_in_maps(inputs)
    res = run_bass_kernel_spmd(nc, in_maps, list(range(N_CORES)))
    out = np.empty((B, T, OUT_SIZE), dtype=np.float32)
    for c in range(N_CORES):
        yt = res.results[c]["yt"]
        out[c * SPB:(c + 1) * SPB] = (
            1.2 * yt[:OUT_SIZE].astype(np.float32)).T.reshape(SPB, T, OUT_SIZE)
    return out


if __name__ == "__main__":
    rng = np.random.default_rng(0)
    demo = {
        "x": rng.standard_normal((B, T, IN_SIZE), dtype=np.float32),
        "Wih_f": rng.standard_normal((2, 120, 257), dtype=np.float32) * 0.1,
        "Whh_f": rng.standard_normal((2, 120, 40), dtype=np.float32) * 0.1,
        "bih_f": rng.standard_normal((2, 120), dtype=np.float32) * 0.1,
        "bhh_f": rng.standard_normal((2, 120), dtype=np.float32) * 0.1,
        "Wih_b": rng.standard_normal((2, 120, 257), dtype=np.float32) * 0.1,
        "Whh_b": rng.standard_normal((2, 120, 40), dtype=np.float32) * 0.1,
        "bih_b": rng.standard_normal((2, 120), dtype=np.float32) * 0.1,
        "bhh_b": rng.standard_normal((2, 120), dtype=np.float32) * 0.1,
        "base_weight": rng.standard_normal((257, 80), dtype=np.float32) * 0.1,
        "spline_weight": rng.standard_normal((257, 80, 8), dtype=np.float32) * 0.1,
        "spline_scaler": np.ones((257, 80), dtype=np.float32),
        "slope": np.ones((257,), dtype=np.float32),
        "lengths": np.full((64,), 1000, dtype=np.int32),
    }
    out = kernel(**demo)
    print("kernel ran:", out.shape, out.dtype, float(out.min()), float(out.max()))
